# revision 84
# baseline (speedup 1.0000x reference)
"""Trainium2 Bass kernel for a GNN message-passing layer.

reference semantics (jax):
    src, dst = edge_index
    messages   = silu(concat(nodes[src], edge_features) @ mw1 + mb1)    # [E, D]
    aggregated = segment_sum(messages, dst, N)                          # [N, D]
    updated    = silu(concat(nodes, aggregated) @ uw1 + ub1) @ uw2 + ub2
    out        = nodes + updated

Distribution: destination-node-tile partition across 8 cores, greedy
load balancing, no collectives. Nodes + weights replicated.

Host prep (layout only, no data-sized float math): edges bucketed by
dst tile; per edge slot the stream carries nodes[src] and ef rows in
fp8 plus a precomputed fp8 scatter one-hot row. Stream is stored
PARTITION-MAJOR (st[d, block*128+col]) so every DMA is 128 rows of
multi-KB contiguous bytes, and the [nsrc|ef] pair of each edge tile is
pre-packed in the DoubleRowSwInterleave weight layout (A/B pairs
interleaved per column, columns reversed) so LDWEIGHTS reads
contiguously (~72ns vs ~103ns plain DoubleRow).

Device pipeline per core (per 8-edge-tile super-chunk; chunks span
node-tile boundaries and flush only at capacity or at 4-tile group
boundaries, so nearly every DVE/ACT op is full width):
  8 msg matmuls (edge pair stationary, DRSwInterleave fp8, wtb moving)
  -> one DVE add of mb1*32 [P,1024] -> one ACT silu (scale 1/32) to
  fp8 msg -> (LAG=3 super-chunks later, one scatter matmul per msg
  matmul emitted after them) fp8 scatter matmuls accumulating agg^T
  for 4 node tiles into one PSUM bank region each. Scatters/copies/
  update groups flow through a closure queue popped between msg
  matmuls so the in-order PE queue never head-blocks on a fresh dep.
  Every 4 node tiles: one batched aggT copy (DVE) + update MLP in
  bf16 (weights stationary, N=512 moving, transposes in bf16, fp32
  residual from natural-layout nodes) carried one chunk later.
  Stream DMA: one piece per super-chunk, alternating gpsimd/sync
  HWDGE rings; group inputs prefetched ~2 tiles early; out stores on
  gpsimd; consts split across sync/scalar rings.

Measured on 8xNC trn2: ~163-166us HW exec (baseline 310.8us), rel err
0.0114 (gate 2e-2). Engine busy ~104-107us each on PE-matmul/DVE/ACT
(three-way balanced); LDWEIGHTS 88us.
"""

import math
import sys

sys.path.insert(0, "/opt/trn_rl_repo")

import ml_dtypes
import numpy as np

import concourse.bacc as bacc
import concourse.mybir as mybir
import concourse.tile as tile
from concourse import bass_utils

P = 128
C = 8  # cores

F32 = mybir.dt.float32
BF16 = mybir.dt.bfloat16
FP8 = mybir.dt.float8e4
AF = mybir.ActivationFunctionType
OP = mybir.AluOpType
PM = mybir.MatmulPerfMode
BF = ml_dtypes.bfloat16
F8 = mybir.dt.np(FP8)
WSC = 32.0  # fp8 weight scale (power of two; undone in the silu scale)


def _tileT(a):
    """[R*P, D] -> [R*D, P] with each 128-row block transposed."""
    R = a.shape[0] // P
    return np.ascontiguousarray(
        a.reshape(R, P, a.shape[1]).transpose(0, 2, 1)
    ).reshape(R * a.shape[1], P)


def _host_prep(nodes, edge_index, edge_features, ntiles_pc):
    """Bucket edges by destination node tile, balance tiles over cores,
    pack [nodes[src] | ef | onehot] pair-block streams."""
    N, D = nodes.shape
    E = edge_index.shape[1]
    N2 = ntiles_pc * P * C
    ntiles = N2 // P

    src = edge_index[0].astype(np.int64)
    dst = edge_index[1].astype(np.int64)
    order = np.argsort(dst // P, kind="stable").astype(np.int64)
    ds = dst[order]
    ss = src[order]

    tileid = ds // P
    counts = np.bincount(tileid, minlength=ntiles)

    # greedy balance: biggest tiles first onto the least-loaded core
    assign = [[] for _ in range(C)]  # global tile ids per core, local order
    loads = np.zeros(C, np.int64)
    for g in np.argsort(-counts, kind="stable"):
        cands = [c for c in range(C) if len(assign[c]) < ntiles_pc]
        c = min(cands, key=lambda c: (loads[c], len(assign[c])))
        assign[c].append(int(g))
        loads[c] += counts[g]
    # per local position, tile counts across cores -> shared trip counts
    pos_counts = np.zeros((C, ntiles_pc), np.int64)
    for c in range(C):
        for p, g in enumerate(assign[c]):
            pos_counts[c, p] = counts[g]
    ktot = [
        int(math.ceil(pos_counts[:, p].max() / P)) for p in range(ntiles_pc)
    ]
    offs = np.zeros(ntiles_pc + 1, np.int64)
    np.cumsum(ktot, out=offs[1:])
    SL = int(offs[-1]) * P  # packed slots per core

    # map: global tile -> (core, local pos)
    t2cp = np.zeros((ntiles, 2), np.int64)
    for c in range(C):
        for p, g in enumerate(assign[c]):
            t2cp[g] = (c, p)

    tile_start = np.zeros(ntiles + 1, np.int64)
    np.cumsum(counts, out=tile_start[1:])
    rank = np.arange(E, dtype=np.int64) - tile_start[tileid]
    core = t2cp[tileid, 0]
    pos = t2cp[tileid, 1]
    slot = offs[pos] * P + rank

    dstoff = np.full((C, SL), -1.0, np.float32)
    dstoff[core, slot] = (ds - tileid * P).astype(np.float32)
    esrc = np.full((C, SL), -1, np.int64)  # edge id feeding each slot
    esrc[core, slot] = order

    ef16 = edge_features.astype(F8)
    n16 = nodes.astype(F8)
    jj = np.arange(P, dtype=np.float32)
    per_core = []
    for c in range(C):
        valid = esrc[c] >= 0
        a = np.zeros((SL, D), F8)  # nodes[src] rows
        b = np.zeros((SL, D), F8)  # ef rows
        eidx = esrc[c][valid]
        a[valid] = n16[src[eidx]]
        b[valid] = ef16[eidx]
        # scatter one-hot rows: oh[e, j] = (dstoff[e] == j); pads (-1) -> 0
        oh = (dstoff[c][:, None] == jj[None, :]).astype(F8)  # [SL, P]
        K = SL // P
        K2 = K // 2
        aT = a.reshape(K, P, D).transpose(0, 2, 1)  # [K, D, P] (d, e)
        bT = b.reshape(K, P, D).transpose(0, 2, 1)
        ohK = oh.reshape(K, P, P)  # natural [e, j] rows
        # per-tile blocks [abI_k | oh_k], stored PARTITION-MAJOR:
        # st[d, block*P + col] so each SBUF partition reads one
        # contiguous run per node tile. abI is the DoubleRowSwInterleave
        # weight layout: per row d, [A127 B127 A126 B126 ... A0 B0].
        ab = np.stack(
            [aT[:, :, ::-1], bT[:, :, ::-1]], axis=3
        ).reshape(K, D, 2 * P)
        blocks = np.stack(
            [ab[:, :, :P], ab[:, :, P:], ohK], axis=1
        ).reshape(K * 3, D, P)
        st = np.ascontiguousarray(
            blocks.transpose(1, 0, 2).reshape(D, K * 3 * P)
        )
        per_core.append(dict(st=st))
    return ktot, assign, per_core


def build_program(N2, D, ntiles_pc, ktot):
    """Build the SPMD Bass program (identical across cores)."""
    assert D == P
    ktot = list(ktot)
    offs = [0]
    for t in range(ntiles_pc):
        offs.append(offs[-1] + ktot[t])
    SL = offs[-1] * P

    nc = bacc.Bacc("TRN2", target_bir_lowering=False, debug=False, num_devices=C)
    NP_ = ntiles_pc * P

    d = lambda name, shape, dt=F32, kind="ExternalInput": nc.dram_tensor(
        name, shape, dt, kind=kind
    ).ap()

    st_d = d("st", [D, (SL // P) * 3 * P], FP8)
    own16_d = d("own16T", [ntiles_pc * D, P], BF16)  # nodes^T bf16 (matmul)
    nat_d = d("own_nat", [NP_, D])  # nodes natural fp32 (residual)
    wtb = d("wtb", [D, 2 * D], FP8)  # [wt*WSC | wb*WSC] pre-cast fp8
    mb8 = d("mb8", [P, 8 * D])  # mb1*WSC tiled
    ua = d("ua", [D, D], BF16)
    ub = d("ub", [D, D], BF16)
    uw2 = d("uw2", [D, D], BF16)
    ub1c = d("ub1c", [P, 1])
    ub2c = d("ub2c", [P, 1])
    ident = d("ident", [P, P], BF16)
    out = d("out_own", [NP_, D], kind="ExternalOutput")

    with tile.TileContext(nc) as tc:
        with (
            tc.tile_pool(name="const", bufs=1) as cp,
            tc.tile_pool(name="sb", bufs=4) as sb,
            tc.tile_pool(name="big", bufs=5) as bigp,
            tc.tile_pool(name="msgp", bufs=6) as mp,
            tc.tile_pool(name="psum2", bufs=1, space="PSUM") as pp,
            tc.tile_pool(name="psumT", bufs=1, space="PSUM") as ppt,
            tc.tile_pool(name="psumM", bufs=2, space="PSUM") as ppm,
            tc.tile_pool(name="psumA", bufs=2, space="PSUM") as ppa,
        ):
            # consts load via the vector/scalar queues so the sync/gpsimd
            # rings start streaming edge data immediately
            def load_const(ap, shape, dt=F32, eng=None):
                t = cp.tile(shape, dt, tag=ap.name)
                (eng or nc.sync).dma_start(out=t[:], in_=ap[:])
                return t

            wtb8 = load_const(wtb, [D, 2 * D], FP8)
            mb8_s = load_const(mb8, [P, 8 * D], eng=nc.scalar)
            ua_s = load_const(ua, [D, D], BF16, eng=nc.scalar)
            ub_s = load_const(ub, [D, D], BF16, eng=nc.scalar)
            uw2_s = load_const(uw2, [D, D], BF16, eng=nc.scalar)
            ub1_s = load_const(ub1c, [P, 1], eng=nc.scalar)
            ub2_s = load_const(ub2c, [P, 1], eng=nc.scalar)
            id_s = load_const(ident, [P, P], BF16, eng=nc.scalar)
            aggT_all = cp.tile([P, ntiles_pc * D], BF16, tag="aggT_all")
            wtb8_r = wtb8[:].rearrange("p (two f) -> p two f", two=2)

            gbufs = {}  # group -> (ownT, nat), DMA'd ~2 node tiles early

            def group_dma(g):
                gw = min(4, ntiles_pc - g * 4)
                W = gw * P
                g0 = g * 4
                ownT = sb.tile([P, 4 * P], BF16, tag="ownT")
                nc.sync.dma_start(
                    out=ownT[:, :W].rearrange("p (j n) -> p j n", n=P),
                    in_=own16_d[g0 * D : (g0 + gw) * D, :].rearrange(
                        "(j d) n -> d j n", d=D
                    ),
                )
                nat = sb.tile([P, 4 * P], F32, tag="nat")
                nc.sync.dma_start(
                    out=nat[:, :W].rearrange("p (j d) -> p j d", d=D),
                    in_=nat_d[g0 * P : (g0 + gw) * P, :].rearrange(
                        "(j p) d -> p j d", p=P
                    ),
                )
                gbufs[g] = (ownT, nat)

            def update_group(g):
                """Stage 3 for node tiles [4g, 4g+4): update MLP + residual."""
                if g not in gbufs:
                    group_dma(g)
                gw = min(4, ntiles_pc - g * 4)
                W = gw * P
                g0 = g * 4
                ownT, nat = gbufs.pop(g)
                ph = pp.tile([P, 4 * P], F32, tag="ph")
                nc.tensor.matmul(
                    out=ph[:, :W], lhsT=ua_s[:], rhs=ownT[:, :W], start=True,
                    stop=False,
                )
                nc.tensor.matmul(
                    out=ph[:, :W],
                    lhsT=ub_s[:],
                    rhs=aggT_all[:, g0 * D : g0 * D + W],
                    start=False,
                    stop=True,
                )
                hT = sb.tile([P, 4 * P], BF16, tag="hT")
                nc.scalar.activation(
                    out=hT[:, :W], in_=ph[:, :W], func=AF.Silu, bias=ub1_s[:, :1]
                )
                po = pp.tile([P, 4 * P], F32, tag="ph")
                nc.tensor.matmul(
                    out=po[:, :W], lhsT=uw2_s[:], rhs=hT[:, :W], start=True, stop=True
                )
                oT = sb.tile([P, 4 * P], BF16, tag="oT")
                # +ub2 on DVE: the Scalar engine is the pacer in group
                # regions; DVE has headroom here
                nc.vector.tensor_scalar_add(
                    out=oT[:, :W], in0=po[:, :W], scalar1=ub2_s[:, :1]
                )
                pOut = ppt.tile([P, 4 * P], BF16, tag="phT")
                for j in range(gw):
                    nc.tensor.transpose(
                        out=pOut[:, j * P : (j + 1) * P],
                        in_=oT[:, j * P : (j + 1) * P],
                        identity=id_s[:],
                    )
                ot = sb.tile([P, 4 * P], F32, tag="ot")
                nc.vector.tensor_tensor(
                    out=ot[:, :W], in0=pOut[:, :W], in1=nat[:, :W], op=OP.add
                )
                # last two groups store via sync (idle once the edge
                # stream ends) so the gpsimd ring's end-of-program drain
                # doesn't serialize behind the final stores
                seng = (
                    nc.sync
                    if g >= math.ceil(ntiles_pc / 4) - 2
                    else nc.gpsimd
                )
                seng.dma_start(
                    out=out[g0 * P : (g0 + gw) * P, :].rearrange(
                        "(j p) d -> p j d", p=P
                    ),
                    in_=ot[:, :W].rearrange("p (j d) -> p j d", d=D),
                )

            # empty (pure-pad) positions never write aggT_all; clear it
            # only when such positions exist (DVE: idle during startup)
            if min(ktot) == 0:
                nc.vector.memset(aggT_all[:], 0)

            # scatter matmuls trail the msg pipeline by LAG chunks and are
            # emitted one-per-msg-matmul so accumulate RMW latency on
            # paggT hides under independent msg matmuls.
            LAG = 3  # in super-chunk (8 edge tile) units
            pagg4_box = [None]  # current group's 4-tile PSUM accumulator
            carry = []  # ops delayed one extra chunk (update groups)
            queue = []  # ready closures (scatter matmuls + aggT copies)
            buf = []  # per-chunk op lists, held back (LAG-1) chunks

            def pop_ops(n):
                for _ in range(n):
                    if queue:
                        queue.pop(0)()

            def push_chunk(ops):
                buf.append(ops)
                if len(buf) >= LAG:
                    queue.extend(buf.pop(0))

            # global chunks: 8 consecutive edge tiles regardless of node
            # tile boundaries, so every DVE add / ACT silu is full-width.
            # chunk slot state:
            cslot = [0]  # next free slot 0..7
            # (pmsg, scatter-maker list, post-chunk closures)
            ctile = [None, None, None]

            def flush_chunk():
                s = cslot[0]
                if not s:
                    return
                W = s * P
                pmsg = ctile[0]
                pop_ops(s)
                msgb = sb.tile([P, 8 * P], BF16, tag="msgb")
                nc.vector.tensor_tensor(
                    out=msgb[:, :W],
                    in0=pmsg[:, :W],
                    in1=mb8_s[:, :W],
                    op=OP.add,
                )
                msg = mp.tile([P, 8 * P], FP8, tag="msg")
                nc.scalar.activation(
                    out=msg[:, :W],
                    in_=msgb[:, :W],
                    func=AF.Silu,
                    scale=1.0 / WSC,
                )
                # bind the msg tile into this chunk's deferred scatters;
                # carried items (update groups) run before them
                ops = carry[:]
                carry.clear()
                for mk in ctile[1]:
                    ops.append(mk(msg))
                carry.extend(ctile[2])
                push_chunk(ops)
                pop_ops(1)
                cslot[0] = 0
                ctile[0] = ctile[1] = ctile[2] = None

            for t in range(ntiles_pc):
                kt = ktot[t]
                if kt:
                    egT = bigp.tile([P, 3 * kt * D], FP8, tag="egT")
                    c0 = offs[t] * 3 * P
                    # one DMA piece per ~8 edge tiles: finer-grained deps
                    for pi, pk in enumerate(range(0, kt, 8)):
                        pw = min(8, kt - pk)
                        lo = pk * 3 * P
                        hi = (pk + pw) * 3 * P
                        eng = nc.gpsimd if (t + pi) % 2 == 0 else nc.sync
                        if t == 0 and pi == 0 and pw > 2:
                            # the first matmul needs only the first two
                            # edge tiles; land them ASAP
                            mid = lo + 2 * 3 * P
                            eng.dma_start(
                                out=egT[:, lo:mid],
                                in_=st_d[:, c0 + lo : c0 + mid],
                            )
                            nc.sync.dma_start(
                                out=egT[:, mid:hi],
                                in_=st_d[:, c0 + mid : c0 + hi],
                            )
                        else:
                            eng.dma_start(
                                out=egT[:, lo:hi],
                                in_=st_d[:, c0 + lo : c0 + hi],
                            )
                    if t % 4 == 0:
                        pagg4 = ppa.tile([P, 4 * D], F32, tag="pagg4")
                        pagg4_box[0] = pagg4
                    t4 = t % 4

                    def scatter_mk(
                        k, j, egT=egT, pagg4=pagg4_box[0], t4=t4, kt=kt
                    ):
                        def mk(msg):
                            def op():
                                nc.tensor.matmul(
                                    out=pagg4[:, t4 * D : (t4 + 1) * D],
                                    lhsT=msg[:, j * P : (j + 1) * P],
                                    rhs=egT[
                                        :, (3 * k + 2) * D : (3 * k + 3) * D
                                    ],
                                    start=(k == 0),
                                    stop=(k == kt - 1),
                                )

                            return op

                        return mk

                    def copy_op(t=t, pagg4=pagg4_box[0]):
                        g0 = (t // 4) * 4
                        # copy maximal runs of non-empty tiles (empty tiles
                        # keep their memset zeros; their PSUM is stale)
                        runs, s = [], None
                        for u in range(g0, t + 1):
                            if ktot[u]:
                                s = u if s is None else s
                            elif s is not None:
                                runs.append((s, u))
                                s = None
                        if s is not None:
                            runs.append((s, t + 1))

                        def op():
                            for a, b in runs:
                                nc.vector.tensor_copy(
                                    out=aggT_all[:, a * D : b * D],
                                    in_=pagg4[
                                        :, (a - g0) * D : (b - g0) * D
                                    ],
                                )

                        return op

                    if t % 4 == 2:
                        group_dma(t // 4)
                    for k in range(kt):
                        if ctile[0] is None:
                            pmsg = ppm.tile([P, 8 * P], F32, tag="pmsg")
                            ctile[0] = pmsg
                            ctile[1] = []
                            ctile[2] = []
                        j = cslot[0]
                        nc.tensor.matmul(
                            out=ctile[0][:, j * P : (j + 1) * P],
                            lhsT=egT[:, 3 * k * D : (3 * k + 2) * D].rearrange(
                                "p (two e) -> p two e", two=2
                            ),
                            rhs=wtb8_r,
                            start=True,
                            stop=True,
                            perf_mode=PM.DoubleRowSwInterleave,
                        )
                        ctile[1].append(scatter_mk(k, j))
                        if k == kt - 1 and (
                            t % 4 == 3 or t == ntiles_pc - 1
                        ):
                            # batched aggT copy right after the last
                            # scatter; the update group one chunk later
                            ctile[1].append(lambda msg, c=copy_op(): c)
                            if t % 4 == 3:
                                ctile[2].append(
                                    lambda g=t // 4: update_group(g)
                                )
                        cslot[0] += 1
                        if cslot[0] == 8 or (
                            k == kt - 1
                            and (t % 4 == 3 or t == ntiles_pc - 1)
                        ):
                            # flush at capacity and at group boundaries so
                            # copies/update groups stay chunk-aligned
                            flush_chunk()
            flush_chunk()
            for ops in buf + [carry]:
                queue.extend(ops)
            buf.clear()
            carry = []
            pop_ops(len(queue))
            if ntiles_pc % 4:
                update_group(ntiles_pc // 4)

    nc.compile()
    return nc


def _run(nc, in_maps, trace=False):
    return bass_utils.run_bass_kernel_spmd(
        nc, in_maps, core_ids=list(range(C)), trace=trace
    )


def make_in_maps(nodes, edge_index, edge_features, mw1, mb1, uw1, ub1, uw2, ub2,
                 ntiles_pc):
    N, D = nodes.shape
    NP_ = ntiles_pc * P
    N2 = NP_ * C
    ktot, assign, per_core = _host_prep(nodes, edge_index, edge_features, ntiles_pc)

    nodes_pad = np.zeros((N2, D), np.float32)
    nodes_pad[:N] = nodes
    ident = np.eye(P, dtype=BF)
    mb8 = np.broadcast_to(
        np.tile(mb1.astype(np.float32) * WSC, 8), (P, 8 * D)
    ).copy()

    shared = dict(
        wtb=np.concatenate(
            [mw1[:D] * WSC, mw1[D:] * WSC], axis=1
        ).astype(F8),
        mb8=mb8,
        ua=np.ascontiguousarray(uw1[:D]).astype(BF),
        ub=np.ascontiguousarray(uw1[D:]).astype(BF),
        uw2=np.ascontiguousarray(uw2).astype(BF),
        ub1c=np.ascontiguousarray(ub1.reshape(D, 1), np.float32),
        ub2c=np.ascontiguousarray(ub2.reshape(D, 1), np.float32),
        ident=ident,
    )
    in_maps = []
    for c in range(C):
        m = dict(shared)
        own = np.concatenate(
            [nodes_pad[g * P : (g + 1) * P] for g in assign[c]], axis=0
        )
        m["own16T"] = _tileT(np.ascontiguousarray(own)).astype(BF)
        m["own_nat"] = np.ascontiguousarray(own, np.float32)
        m["st"] = per_core[c]["st"]
        in_maps.append(m)
    return ktot, assign, in_maps


def kernel(nodes, edge_index, edge_features, mw1, mb1, uw1, ub1, uw2, ub2):
    nodes = np.asarray(nodes, np.float32)
    edge_index = np.asarray(edge_index, np.int32)
    edge_features = np.asarray(edge_features, np.float32)
    N, D = nodes.shape
    ntiles_pc = math.ceil(N / (C * P))
    ktot, assign, in_maps = make_in_maps(
        nodes, edge_index, edge_features, mw1, mb1, uw1, ub1, uw2, ub2, ntiles_pc
    )
    N2 = ntiles_pc * P * C
    nc = build_program(N2, D, ntiles_pc, ktot)
    res = _run(nc, in_maps)
    out_full = np.zeros((N2, D), np.float32)
    for c in range(C):
        oc = res.results[c]["out_own"]
        for p, g in enumerate(assign[c]):
            out_full[g * P : (g + 1) * P] = oc[p * P : (p + 1) * P]
    return out_full[:N].astype(np.float32)


if __name__ == "__main__":
    rng = np.random.default_rng(0)
    N, E, D = 4096, 16384, 128
    nodes = rng.standard_normal((N, D), dtype=np.float32)
    edge_index = rng.integers(0, N, (2, E)).astype(np.int32)
    ef = rng.standard_normal((E, D), dtype=np.float32)
    s2, s1 = 1 / np.sqrt(2 * D), 1 / np.sqrt(D)
    mw1 = rng.uniform(-s2, s2, (2 * D, D)).astype(np.float32)
    mb1 = rng.uniform(-s2, s2, D).astype(np.float32)
    uw1 = rng.uniform(-s2, s2, (2 * D, D)).astype(np.float32)
    ub1 = rng.uniform(-s2, s2, D).astype(np.float32)
    uw2 = rng.uniform(-s1, s1, (D, D)).astype(np.float32)
    ub2 = rng.uniform(-s1, s1, D).astype(np.float32)

    def silu(x):
        return x / (1 + np.exp(-x))

    def ref():
        src, dst = edge_index
        msg = silu(np.concatenate([nodes[src], ef], 1) @ mw1 + mb1)
        agg = np.zeros((N, D), np.float32)
        np.add.at(agg, dst, msg)
        upd = silu(np.concatenate([nodes, agg], 1) @ uw1 + ub1) @ uw2 + ub2
        return nodes + upd

    out = kernel(nodes, edge_index, ef, mw1, mb1, uw1, ub1, uw2, ub2)
    exp = ref()
    err = np.abs(out - exp).max() / np.abs(exp).max()
    print("tiny rel err:", err)


# revision 85
# speedup vs baseline: 1.0901x; 1.0901x over previous
"""Trainium2 Bass kernel for a GNN message-passing layer.

reference semantics (jax):
    src, dst = edge_index
    messages   = silu(concat(nodes[src], edge_features) @ mw1 + mb1)    # [E, D]
    aggregated = segment_sum(messages, dst, N)                          # [N, D]
    updated    = silu(concat(nodes, aggregated) @ uw1 + ub1) @ uw2 + ub2
    out        = nodes + updated

Distribution: destination-node-tile partition across 8 cores, greedy
load balancing, no collectives. Nodes + weights replicated.

Host prep (layout only, no data-sized float math): edges bucketed by
dst tile; per edge slot the stream carries nodes[src] and ef rows in
fp8 plus a precomputed fp8 scatter one-hot row. Stream is stored
PARTITION-MAJOR (st[d, block*128+col]) so every DMA is 128 rows of
multi-KB contiguous bytes, and the [nsrc|ef] pair of each edge tile is
pre-packed in the DoubleRowSwInterleave weight layout (A/B pairs
interleaved per column, columns reversed) so LDWEIGHTS reads
contiguously (~72ns vs ~103ns plain DoubleRow).

Device pipeline per core (per 8-edge-tile super-chunk; chunks span
node-tile boundaries and flush only at capacity or at 4-tile group
boundaries, so nearly every DVE/ACT op is full width):
  8 msg matmuls (edge pair stationary, DRSwInterleave fp8, wtb moving)
  -> one DVE add of mb1*32 [P,1024] -> one ACT silu (scale 1/32) to
  fp8 msg -> (LAG=3 super-chunks later, one scatter matmul per msg
  matmul emitted after them) fp8 scatter matmuls accumulating agg^T
  for 4 node tiles into one PSUM bank region each. Scatters/copies/
  update groups flow through a closure queue popped between msg
  matmuls so the in-order PE queue never head-blocks on a fresh dep.
  Every 4 node tiles: one batched aggT copy (DVE) + update MLP in
  bf16 (weights stationary, N=512 moving, transposes in bf16, fp32
  residual from natural-layout nodes) carried one chunk later.
  Stream DMA: one piece per super-chunk, alternating gpsimd/sync
  HWDGE rings; group inputs prefetched ~2 tiles early; out stores on
  gpsimd; consts split across sync/scalar rings.

Measured on 8xNC trn2: ~163-166us HW exec (baseline 310.8us), rel err
0.0114 (gate 2e-2). Engine busy ~104-107us each on PE-matmul/DVE/ACT
(three-way balanced); LDWEIGHTS 88us.
"""

import math
import sys

sys.path.insert(0, "/opt/trn_rl_repo")

import ml_dtypes
import numpy as np

import concourse.bacc as bacc
import concourse.mybir as mybir
import concourse.tile as tile
from concourse import bass_utils

P = 128
C = 8  # cores

F32 = mybir.dt.float32
BF16 = mybir.dt.bfloat16
FP8 = mybir.dt.float8e4
AF = mybir.ActivationFunctionType
OP = mybir.AluOpType
PM = mybir.MatmulPerfMode
BF = ml_dtypes.bfloat16
F8 = mybir.dt.np(FP8)
WSC = 32.0  # fp8 weight scale (power of two; undone in the silu scale)


def _tileT(a):
    """[R*P, D] -> [R*D, P] with each 128-row block transposed."""
    R = a.shape[0] // P
    return np.ascontiguousarray(
        a.reshape(R, P, a.shape[1]).transpose(0, 2, 1)
    ).reshape(R * a.shape[1], P)


def _host_prep(nodes, edge_index, edge_features, ntiles_pc):
    """Bucket edges by destination node tile, balance tiles over cores,
    pack [nodes[src] | ef | onehot] pair-block streams."""
    N, D = nodes.shape
    E = edge_index.shape[1]
    N2 = ntiles_pc * P * C
    ntiles = N2 // P

    src = edge_index[0].astype(np.int64)
    dst = edge_index[1].astype(np.int64)
    order = np.argsort(dst // P, kind="stable").astype(np.int64)
    ds = dst[order]
    ss = src[order]

    tileid = ds // P
    counts = np.bincount(tileid, minlength=ntiles)

    # greedy balance: biggest tiles first onto the least-loaded core
    assign = [[] for _ in range(C)]  # global tile ids per core, local order
    loads = np.zeros(C, np.int64)
    for g in np.argsort(-counts, kind="stable"):
        cands = [c for c in range(C) if len(assign[c]) < ntiles_pc]
        c = min(cands, key=lambda c: (loads[c], len(assign[c])))
        assign[c].append(int(g))
        loads[c] += counts[g]
    # per local position, tile counts across cores -> shared trip counts
    pos_counts = np.zeros((C, ntiles_pc), np.int64)
    for c in range(C):
        for p, g in enumerate(assign[c]):
            pos_counts[c, p] = counts[g]
    ktot = [
        int(math.ceil(pos_counts[:, p].max() / P)) for p in range(ntiles_pc)
    ]
    offs = np.zeros(ntiles_pc + 1, np.int64)
    np.cumsum(ktot, out=offs[1:])
    SL = int(offs[-1]) * P  # packed slots per core

    # map: global tile -> (core, local pos)
    t2cp = np.zeros((ntiles, 2), np.int64)
    for c in range(C):
        for p, g in enumerate(assign[c]):
            t2cp[g] = (c, p)

    tile_start = np.zeros(ntiles + 1, np.int64)
    np.cumsum(counts, out=tile_start[1:])
    rank = np.arange(E, dtype=np.int64) - tile_start[tileid]
    core = t2cp[tileid, 0]
    pos = t2cp[tileid, 1]
    slot = offs[pos] * P + rank

    dstoff = np.full((C, SL), -1.0, np.float32)
    dstoff[core, slot] = (ds - tileid * P).astype(np.float32)
    esrc = np.full((C, SL), -1, np.int64)  # edge id feeding each slot
    esrc[core, slot] = order

    ef16 = edge_features.astype(F8)
    n16 = nodes.astype(F8)
    jj = np.arange(P, dtype=np.float32)
    per_core = []
    for c in range(C):
        valid = esrc[c] >= 0
        a = np.zeros((SL, D), F8)  # nodes[src] rows
        b = np.zeros((SL, D), F8)  # ef rows
        eidx = esrc[c][valid]
        a[valid] = n16[src[eidx]]
        b[valid] = ef16[eidx]
        # scatter one-hot rows: oh[e, j] = (dstoff[e] == j); pads (-1) -> 0
        oh = (dstoff[c][:, None] == jj[None, :]).astype(F8)  # [SL, P]
        K = SL // P
        K2 = K // 2
        aT = a.reshape(K, P, D).transpose(0, 2, 1)  # [K, D, P] (d, e)
        bT = b.reshape(K, P, D).transpose(0, 2, 1)
        ohK = oh.reshape(K, P, P)  # natural [e, j] rows
        # per-tile blocks [abI_k | oh_k], stored PARTITION-MAJOR:
        # st[d, block*P + col] so each SBUF partition reads one
        # contiguous run per node tile. abI is the DoubleRowSwInterleave
        # weight layout: per row d, [A127 B127 A126 B126 ... A0 B0].
        ab = np.stack(
            [aT[:, :, ::-1], bT[:, :, ::-1]], axis=3
        ).reshape(K, D, 2 * P)
        blocks = np.stack(
            [ab[:, :, :P], ab[:, :, P:], ohK], axis=1
        ).reshape(K * 3, D, P)
        st = np.ascontiguousarray(
            blocks.transpose(1, 0, 2).reshape(D, K * 3 * P)
        )
        per_core.append(dict(st=st))
    return ktot, assign, per_core


def build_program(N2, D, ntiles_pc, ktot):
    """Build the SPMD Bass program (identical across cores)."""
    assert D == P
    ktot = list(ktot)
    offs = [0]
    for t in range(ntiles_pc):
        offs.append(offs[-1] + ktot[t])
    SL = offs[-1] * P

    nc = bacc.Bacc("TRN2", target_bir_lowering=False, debug=False, num_devices=C)
    NP_ = ntiles_pc * P

    d = lambda name, shape, dt=F32, kind="ExternalInput": nc.dram_tensor(
        name, shape, dt, kind=kind
    ).ap()

    st_d = d("st", [D, (SL // P) * 3 * P], FP8)
    own16_d = d("own16T", [ntiles_pc * D, P], BF16)  # nodes^T bf16 (matmul)
    nat_d = d("own_nat", [NP_, D])  # nodes natural fp32 (residual)
    wtb = d("wtb", [D, 2 * D], FP8)  # [wt*WSC | wb*WSC] pre-cast fp8
    mb8 = d("mb8", [P, 8 * D])  # mb1*WSC tiled
    ua = d("ua", [D, D], BF16)
    ub = d("ub", [D, D], BF16)
    uw2 = d("uw2", [D, D], BF16)
    ub1c = d("ub1c", [P, 1])
    ub2c = d("ub2c", [P, 1])
    ident = d("ident", [P, P], BF16)
    out = d("out_own", [NP_, D], kind="ExternalOutput")

    with tile.TileContext(nc) as tc:
        with (
            tc.tile_pool(name="const", bufs=1) as cp,
            tc.tile_pool(name="sb", bufs=4) as sb,
            tc.tile_pool(name="big", bufs=5) as bigp,
            tc.tile_pool(name="msgp", bufs=6) as mp,
            tc.tile_pool(name="psum2", bufs=1, space="PSUM") as pp,
            tc.tile_pool(name="psumT", bufs=1, space="PSUM") as ppt,
            tc.tile_pool(name="psumM", bufs=2, space="PSUM") as ppm,
            tc.tile_pool(name="psumA", bufs=2, space="PSUM") as ppa,
        ):
            # consts load via the vector/scalar queues so the sync/gpsimd
            # rings start streaming edge data immediately
            def load_const(ap, shape, dt=F32, eng=None):
                t = cp.tile(shape, dt, tag=ap.name)
                (eng or nc.sync).dma_start(out=t[:], in_=ap[:])
                return t

            wtb8 = load_const(wtb, [D, 2 * D], FP8)
            mb8_s = load_const(mb8, [P, 8 * D], eng=nc.scalar)
            ua_s = load_const(ua, [D, D], BF16, eng=nc.scalar)
            ub_s = load_const(ub, [D, D], BF16, eng=nc.scalar)
            uw2_s = load_const(uw2, [D, D], BF16, eng=nc.scalar)
            ub1_s = load_const(ub1c, [P, 1], eng=nc.scalar)
            ub2_s = load_const(ub2c, [P, 1], eng=nc.scalar)
            id_s = load_const(ident, [P, P], BF16, eng=nc.scalar)
            aggT_all = cp.tile([P, ntiles_pc * D], BF16, tag="aggT_all")
            wtb8_r = wtb8[:].rearrange("p (two f) -> p two f", two=2)

            gbufs = {}  # group -> (ownT, nat), DMA'd ~2 node tiles early

            def group_dma(g):
                gw = min(4, ntiles_pc - g * 4)
                W = gw * P
                g0 = g * 4
                ownT = sb.tile([P, 4 * P], BF16, tag="ownT")
                nc.sync.dma_start(
                    out=ownT[:, :W].rearrange("p (j n) -> p j n", n=P),
                    in_=own16_d[g0 * D : (g0 + gw) * D, :].rearrange(
                        "(j d) n -> d j n", d=D
                    ),
                )
                nat = sb.tile([P, 4 * P], F32, tag="nat")
                nc.sync.dma_start(
                    out=nat[:, :W].rearrange("p (j d) -> p j d", d=D),
                    in_=nat_d[g0 * P : (g0 + gw) * P, :].rearrange(
                        "(j p) d -> p j d", p=P
                    ),
                )
                gbufs[g] = (ownT, nat)

            def update_group(g):
                """Stage 3 for node tiles [4g, 4g+4): update MLP + residual."""
                if g not in gbufs:
                    group_dma(g)
                gw = min(4, ntiles_pc - g * 4)
                W = gw * P
                g0 = g * 4
                ownT, nat = gbufs.pop(g)
                ph = pp.tile([P, 4 * P], F32, tag="ph")
                nc.tensor.matmul(
                    out=ph[:, :W], lhsT=ua_s[:], rhs=ownT[:, :W], start=True,
                    stop=False,
                )
                nc.tensor.matmul(
                    out=ph[:, :W],
                    lhsT=ub_s[:],
                    rhs=aggT_all[:, g0 * D : g0 * D + W],
                    start=False,
                    stop=True,
                )
                hT = sb.tile([P, 4 * P], BF16, tag="hT")
                nc.scalar.activation(
                    out=hT[:, :W], in_=ph[:, :W], func=AF.Silu, bias=ub1_s[:, :1]
                )
                po = pp.tile([P, 4 * P], F32, tag="ph")
                nc.tensor.matmul(
                    out=po[:, :W], lhsT=uw2_s[:], rhs=hT[:, :W], start=True, stop=True
                )
                oT = sb.tile([P, 4 * P], BF16, tag="oT")
                nc.scalar.activation(
                    out=oT[:, :W], in_=po[:, :W], func=AF.Identity, bias=ub2_s[:, :1]
                )
                pOut = ppt.tile([P, 4 * P], BF16, tag="phT")
                for j in range(gw):
                    nc.tensor.transpose(
                        out=pOut[:, j * P : (j + 1) * P],
                        in_=oT[:, j * P : (j + 1) * P],
                        identity=id_s[:],
                    )
                ot = sb.tile([P, 4 * P], F32, tag="ot")
                nc.vector.tensor_tensor(
                    out=ot[:, :W], in0=pOut[:, :W], in1=nat[:, :W], op=OP.add
                )
                # last two groups store via sync (idle once the edge
                # stream ends) so the gpsimd ring's end-of-program drain
                # doesn't serialize behind the final stores
                seng = (
                    nc.sync
                    if g >= math.ceil(ntiles_pc / 4) - 2
                    else nc.gpsimd
                )
                seng.dma_start(
                    out=out[g0 * P : (g0 + gw) * P, :].rearrange(
                        "(j p) d -> p j d", p=P
                    ),
                    in_=ot[:, :W].rearrange("p (j d) -> p j d", d=D),
                )

            # empty (pure-pad) positions never write aggT_all; clear it
            # only when such positions exist (DVE: idle during startup)
            if min(ktot) == 0:
                nc.vector.memset(aggT_all[:], 0)

            # scatter matmuls trail the msg pipeline by LAG chunks and are
            # emitted one-per-msg-matmul so accumulate RMW latency on
            # paggT hides under independent msg matmuls.
            LAG = 3  # in super-chunk (8 edge tile) units
            pagg4_box = [None]  # current group's 4-tile PSUM accumulator
            carry = []  # ops delayed one extra chunk (update groups)
            queue = []  # ready closures (scatter matmuls + aggT copies)
            buf = []  # per-chunk op lists, held back (LAG-1) chunks

            def pop_ops(n):
                for _ in range(n):
                    if queue:
                        queue.pop(0)()

            def push_chunk(ops):
                buf.append(ops)
                if len(buf) >= LAG:
                    queue.extend(buf.pop(0))

            # global chunks: 8 consecutive edge tiles regardless of node
            # tile boundaries, so every DVE add / ACT silu is full-width.
            # chunk slot state:
            cslot = [0]  # next free slot 0..7
            # (pmsg, scatter-maker list, post-chunk closures)
            ctile = [None, None, None]

            def flush_chunk():
                s = cslot[0]
                if not s:
                    return
                W = s * P
                pmsg = ctile[0]
                pop_ops(s)
                msgb = sb.tile([P, 8 * P], BF16, tag="msgb")
                nc.vector.tensor_tensor(
                    out=msgb[:, :W],
                    in0=pmsg[:, :W],
                    in1=mb8_s[:, :W],
                    op=OP.add,
                )
                msg = mp.tile([P, 8 * P], FP8, tag="msg")
                nc.scalar.activation(
                    out=msg[:, :W],
                    in_=msgb[:, :W],
                    func=AF.Silu,
                    scale=1.0 / WSC,
                )
                # bind the msg tile into this chunk's deferred scatters;
                # carried items (update groups) run before them
                ops = carry[:]
                carry.clear()
                for mk in ctile[1]:
                    ops.append(mk(msg))
                carry.extend(ctile[2])
                push_chunk(ops)
                pop_ops(1)
                cslot[0] = 0
                ctile[0] = ctile[1] = ctile[2] = None

            for t in range(ntiles_pc):
                kt = ktot[t]
                if kt:
                    egT = bigp.tile([P, 3 * kt * D], FP8, tag="egT")
                    c0 = offs[t] * 3 * P
                    # one DMA piece per ~8 edge tiles: finer-grained deps
                    for pi, pk in enumerate(range(0, kt, 8)):
                        pw = min(8, kt - pk)
                        lo = pk * 3 * P
                        hi = (pk + pw) * 3 * P
                        eng = nc.gpsimd if (t + pi) % 2 == 0 else nc.sync
                        if t == 0 and pi == 0 and pw > 2:
                            # the first matmul needs only the first two
                            # edge tiles; land them ASAP
                            mid = lo + 2 * 3 * P
                            eng.dma_start(
                                out=egT[:, lo:mid],
                                in_=st_d[:, c0 + lo : c0 + mid],
                            )
                            nc.sync.dma_start(
                                out=egT[:, mid:hi],
                                in_=st_d[:, c0 + mid : c0 + hi],
                            )
                        else:
                            eng.dma_start(
                                out=egT[:, lo:hi],
                                in_=st_d[:, c0 + lo : c0 + hi],
                            )
                    if t % 4 == 0:
                        pagg4 = ppa.tile([P, 4 * D], F32, tag="pagg4")
                        pagg4_box[0] = pagg4
                    t4 = t % 4

                    def scatter_mk(
                        k, j, egT=egT, pagg4=pagg4_box[0], t4=t4, kt=kt
                    ):
                        def mk(msg):
                            def op():
                                nc.tensor.matmul(
                                    out=pagg4[:, t4 * D : (t4 + 1) * D],
                                    lhsT=msg[:, j * P : (j + 1) * P],
                                    rhs=egT[
                                        :, (3 * k + 2) * D : (3 * k + 3) * D
                                    ],
                                    start=(k == 0),
                                    stop=(k == kt - 1),
                                )

                            return op

                        return mk

                    def copy_op(t=t, pagg4=pagg4_box[0]):
                        g0 = (t // 4) * 4
                        # copy maximal runs of non-empty tiles (empty tiles
                        # keep their memset zeros; their PSUM is stale)
                        runs, s = [], None
                        for u in range(g0, t + 1):
                            if ktot[u]:
                                s = u if s is None else s
                            elif s is not None:
                                runs.append((s, u))
                                s = None
                        if s is not None:
                            runs.append((s, t + 1))

                        def op():
                            for a, b in runs:
                                nc.vector.tensor_copy(
                                    out=aggT_all[:, a * D : b * D],
                                    in_=pagg4[
                                        :, (a - g0) * D : (b - g0) * D
                                    ],
                                )

                        return op

                    if t % 4 == 2:
                        group_dma(t // 4)
                    for k in range(kt):
                        if ctile[0] is None:
                            pmsg = ppm.tile([P, 8 * P], F32, tag="pmsg")
                            ctile[0] = pmsg
                            ctile[1] = []
                            ctile[2] = []
                        j = cslot[0]
                        nc.tensor.matmul(
                            out=ctile[0][:, j * P : (j + 1) * P],
                            lhsT=egT[:, 3 * k * D : (3 * k + 2) * D].rearrange(
                                "p (two e) -> p two e", two=2
                            ),
                            rhs=wtb8_r,
                            start=True,
                            stop=True,
                            perf_mode=PM.DoubleRowSwInterleave,
                        )
                        ctile[1].append(scatter_mk(k, j))
                        if k == kt - 1 and (
                            t % 4 == 3 or t == ntiles_pc - 1
                        ):
                            # batched aggT copy right after the last
                            # scatter; the update group one chunk later
                            ctile[1].append(lambda msg, c=copy_op(): c)
                            if t % 4 == 3:
                                ctile[2].append(
                                    lambda g=t // 4: update_group(g)
                                )
                        cslot[0] += 1
                        if cslot[0] == 8 or (
                            k == kt - 1
                            and (t % 4 == 3 or t == ntiles_pc - 1)
                        ):
                            # flush at capacity and at group boundaries so
                            # copies/update groups stay chunk-aligned
                            flush_chunk()
            flush_chunk()
            for ops in buf + [carry]:
                queue.extend(ops)
            buf.clear()
            carry = []
            pop_ops(len(queue))
            if ntiles_pc % 4:
                update_group(ntiles_pc // 4)

    nc.compile()
    return nc


def _run(nc, in_maps, trace=False):
    return bass_utils.run_bass_kernel_spmd(
        nc, in_maps, core_ids=list(range(C)), trace=trace
    )


def make_in_maps(nodes, edge_index, edge_features, mw1, mb1, uw1, ub1, uw2, ub2,
                 ntiles_pc):
    N, D = nodes.shape
    NP_ = ntiles_pc * P
    N2 = NP_ * C
    ktot, assign, per_core = _host_prep(nodes, edge_index, edge_features, ntiles_pc)

    nodes_pad = np.zeros((N2, D), np.float32)
    nodes_pad[:N] = nodes
    ident = np.eye(P, dtype=BF)
    mb8 = np.broadcast_to(
        np.tile(mb1.astype(np.float32) * WSC, 8), (P, 8 * D)
    ).copy()

    shared = dict(
        wtb=np.concatenate(
            [mw1[:D] * WSC, mw1[D:] * WSC], axis=1
        ).astype(F8),
        mb8=mb8,
        ua=np.ascontiguousarray(uw1[:D]).astype(BF),
        ub=np.ascontiguousarray(uw1[D:]).astype(BF),
        uw2=np.ascontiguousarray(uw2).astype(BF),
        ub1c=np.ascontiguousarray(ub1.reshape(D, 1), np.float32),
        ub2c=np.ascontiguousarray(ub2.reshape(D, 1), np.float32),
        ident=ident,
    )
    in_maps = []
    for c in range(C):
        m = dict(shared)
        own = np.concatenate(
            [nodes_pad[g * P : (g + 1) * P] for g in assign[c]], axis=0
        )
        m["own16T"] = _tileT(np.ascontiguousarray(own)).astype(BF)
        m["own_nat"] = np.ascontiguousarray(own, np.float32)
        m["st"] = per_core[c]["st"]
        in_maps.append(m)
    return ktot, assign, in_maps


def kernel(nodes, edge_index, edge_features, mw1, mb1, uw1, ub1, uw2, ub2):
    nodes = np.asarray(nodes, np.float32)
    edge_index = np.asarray(edge_index, np.int32)
    edge_features = np.asarray(edge_features, np.float32)
    N, D = nodes.shape
    ntiles_pc = math.ceil(N / (C * P))
    ktot, assign, in_maps = make_in_maps(
        nodes, edge_index, edge_features, mw1, mb1, uw1, ub1, uw2, ub2, ntiles_pc
    )
    N2 = ntiles_pc * P * C
    nc = build_program(N2, D, ntiles_pc, ktot)
    res = _run(nc, in_maps)
    out_full = np.zeros((N2, D), np.float32)
    for c in range(C):
        oc = res.results[c]["out_own"]
        for p, g in enumerate(assign[c]):
            out_full[g * P : (g + 1) * P] = oc[p * P : (p + 1) * P]
    return out_full[:N].astype(np.float32)


if __name__ == "__main__":
    rng = np.random.default_rng(0)
    N, E, D = 4096, 16384, 128
    nodes = rng.standard_normal((N, D), dtype=np.float32)
    edge_index = rng.integers(0, N, (2, E)).astype(np.int32)
    ef = rng.standard_normal((E, D), dtype=np.float32)
    s2, s1 = 1 / np.sqrt(2 * D), 1 / np.sqrt(D)
    mw1 = rng.uniform(-s2, s2, (2 * D, D)).astype(np.float32)
    mb1 = rng.uniform(-s2, s2, D).astype(np.float32)
    uw1 = rng.uniform(-s2, s2, (2 * D, D)).astype(np.float32)
    ub1 = rng.uniform(-s2, s2, D).astype(np.float32)
    uw2 = rng.uniform(-s1, s1, (D, D)).astype(np.float32)
    ub2 = rng.uniform(-s1, s1, D).astype(np.float32)

    def silu(x):
        return x / (1 + np.exp(-x))

    def ref():
        src, dst = edge_index
        msg = silu(np.concatenate([nodes[src], ef], 1) @ mw1 + mb1)
        agg = np.zeros((N, D), np.float32)
        np.add.at(agg, dst, msg)
        upd = silu(np.concatenate([nodes, agg], 1) @ uw1 + ub1) @ uw2 + ub2
        return nodes + upd

    out = kernel(nodes, edge_index, ef, mw1, mb1, uw1, ub1, uw2, ub2)
    exp = ref()
    err = np.abs(out - exp).max() / np.abs(exp).max()
    print("tiny rel err:", err)
